# revision 21
# baseline (speedup 1.0000x reference)
"""APIQMixer Trainium2 kernel — 8-core data-parallel over the b*t axis.

Layout strategy (per core, nbt=2048 rows):
  - Everything on-chip is kept "transposed": features on SBUF partitions,
    bt rows on the free dim.  Host passes statesT/qvT/qvB so every DMA is
    a contiguous row load (fp32 DMA-transpose is unsupported on TRN2).
  - L1 hypernet matmuls (K=32) are row-packed 4x into the PE array
    (tile_position=(32r,0)), one [128,1024] 2-bank PSUM tile per agent-pair.
  - ReLU = the only PSUM->SBUF pass, split between ScalarE and VectorE.
  - All agent-sums (b_a+b_e, w_e, l2a, l2e) accumulate in PSUM via the PE
    (start/stop accumulation groups), never through the vector engines.
  - matmul operands are bitcast to float32r (1 cycle/row at N>=512 vs 4 for
    plain fp32).
"""

import numpy as np

import concourse.bass as bass
from concourse import bacc
import concourse.mybir as mybir
from concourse.bass import ds, ts
from concourse.bass_utils import run_bass_kernel_spmd
from concourse.tile import TileContext

F32 = mybir.dt.float32
F32R = mybir.dt.float32r
BF16 = mybir.dt.bfloat16
AF = mybir.ActivationFunctionType
OP = mybir.AluOpType

A, NE = 10, 11
FA = FE = 32
E, H = 64, 256          # E = mixing dim, H = hypernet hidden
B, T = 128, 128
BT = B * T
SD = A * FA + NE * FE   # 672
NCORES = 8
NBT = BT // NCORES      # 2048 rows per core
NF = 512                # free-dim tile (bt rows per tile)
NT = NBT // NF          # 4 tiles
NAG = A + NE            # 21 agents
NSTACK = 6              # ceil(21/4) input stacks of 4 agents

W_NAMES = [
    "l1a_w1", "l1a_w2", "l1e_w1", "l1e_w2",
    "l2a_w1", "l2a_w2", "l2e_w1", "l2e_w2",
]


def _mm(nc, out, lhsT, rhs, **kw):
    nc.tensor.matmul(out, lhsT, rhs, **kw)


def build(fuse_wabs=True):
    nc = bacc.Bacc()

    qvT_e = nc.declare_dram_parameter("qvT", [A, NBT], BF16, isOutput=False)
    qvB_e = nc.declare_dram_parameter("qvB", [A, E, NBT], F32, isOutput=False)
    stT_e = nc.declare_dram_parameter("statesT", [SD, NBT], BF16, isOutput=False)
    # host-prepped weight layouts (one DMA each):
    #   w1x4:  [128, 512] = 4 row-replicas of [l1x_w1 | l2x_w1]
    #   w2l1a: [128, 256] = l1a_w2 two K-chunks side by side ([w|b] cols)
    #   w2l1e: [128, 256] = l1e_w2 two K-chunks, cols reordered to [b|w]
    #   w2l2x: [128, 130] = l2x_w2 two K-chunks ([e-cols|bias] cols)
    #   bvec:  [128, 14] packed bias folds
    #   cmat:  [128, 257] = fmat2 | ones10 | dotsel
    w1a4_e = nc.declare_dram_parameter("w1a4", [128, 2 * H], BF16, isOutput=False)
    w1e4_e = nc.declare_dram_parameter("w1e4", [128, 2 * H], BF16, isOutput=False)
    w2l1a_e = nc.declare_dram_parameter("w2l1a", [128, 256], BF16, isOutput=False)
    w2l1e_e = nc.declare_dram_parameter("w2l1e", [128, 256], BF16, isOutput=False)
    w2l2a_e = nc.declare_dram_parameter("w2l2a", [128, 2 * (E + 1)], BF16, isOutput=False)
    w2l2e_e = nc.declare_dram_parameter("w2l2e", [128, 2 * (E + 1)], BF16, isOutput=False)
    bvec_e = nc.declare_dram_parameter("bvec", [128, 14], F32, isOutput=False)
    cmat_e = nc.declare_dram_parameter("cmat", [128, 321], BF16, isOutput=False)
    out_e = nc.declare_dram_parameter("out", [NBT], F32, isOutput=True)

    from contextlib import ExitStack
    with TileContext(nc) as tc, ExitStack() as ctx:
        const = ctx.enter_context(tc.tile_pool(name="const", bufs=1))
        hpool = ctx.enter_context(tc.tile_pool(name="hpool", bufs=7))
        qbp = ctx.enter_context(tc.tile_pool(name="qbp", bufs=3))
        mix = ctx.enter_context(tc.tile_pool(name="mix", bufs=2))
        ph = ctx.enter_context(tc.tile_pool(name="ph", bufs=4, space="PSUM"))
        pt = ctx.enter_context(tc.tile_pool(name="pt", bufs=4, space="PSUM"))

        # ---------------- static loads ----------------
        stacks = []
        for s in range(NSTACK):
            rows = min(128, SD - 128 * s)
            st = const.tile([rows, NBT], BF16, name=f"stack{s}")
            nc.sync.dma_start(out=st[:, :], in_=stT_e[ds(128 * s, rows), :])
            stacks.append(st)

        w1a4 = const.tile([128, 2 * H], BF16, name="w1a4")
        nc.sync.dma_start(out=w1a4[:, :], in_=w1a4_e[:, :])
        w1e4 = const.tile([128, 2 * H], BF16, name="w1e4")
        nc.sync.dma_start(out=w1e4[:, :], in_=w1e4_e[:, :])
        w2l1a = const.tile([128, 256], BF16, name="w2l1a")
        nc.sync.dma_start(out=w2l1a[:, :], in_=w2l1a_e[:, :])
        w2l1e = const.tile([128, 256], BF16, name="w2l1e")
        nc.sync.dma_start(out=w2l1e[:, :], in_=w2l1e_e[:, :])
        w2l2a = const.tile([128, 2 * (E + 1)], BF16, name="w2l2a")
        nc.sync.dma_start(out=w2l2a[:, :], in_=w2l2a_e[:, :])
        w2l2e = const.tile([128, 2 * (E + 1)], BF16, name="w2l2e")
        nc.sync.dma_start(out=w2l2e[:, :], in_=w2l2e_e[:, :])

        qvT = const.tile([A, NBT], BF16, name="qvT")
        nc.sync.dma_start(out=qvT[:, :], in_=qvT_e[:, :])

        cmat = const.tile([128, 321], BF16, name="cmat")
        nc.sync.dma_start(out=cmat[:, :], in_=cmat_e[:, :])
        fmat2 = cmat[:, 0:E]            # rows 64:128 = I (select upper half)
        fmat = cmat[:, E:2 * E]         # both halves = I (sum halves)
        ones10 = cmat[0:A, 128:256]     # [10,128] all ones
        dotsel = cmat[0:E, 256:321]     # [64,65]: col 64 = ones

        bvec = const.tile([128, 14], F32, name="bvec")
        nc.sync.dma_start(out=bvec[:, :], in_=bvec_e[:, :])
        b1a_sb = bvec[:, 0:4]
        b1e_sb = bvec[:, 4:8]
        wab_sb = bvec[:, 8:9]
        zb_sb = bvec[0:E, 9:10]
        web_sb = bvec[:, 10:11]       # rows 64:128 used
        w2ab_sb = bvec[0:E, 11:12]
        w2eb_sb = bvec[0:E, 12:13]
        ob_sb = bvec[:, 13:14]        # row 64 used

        # hard sync point after constant loads so loop matmuls don't
        # accumulate per-DMA waits
        tc.strict_bb_all_engine_barrier()

        relu_ctr = [0]

        def relu_op(dst, src, bias_ap):
            # alternate ScalarE (3/5) and VectorE (2/5)
            i = relu_ctr[0] % 5
            relu_ctr[0] += 1
            if i < 3:
                nc.scalar.activation(dst, src, AF.Relu, bias=bias_ap)
            else:
                nc.vector.tensor_scalar(dst, src, bias_ap, 0.0, OP.add, OP.max)

        # ---------------- main loop over bt tiles ----------------
        for t in range(NT):
            btsl = ds(NF * t, NF)
            # persistent per-tile accumulators (each 1 PSUM bank)
            hidacc = None
            # binary-counter DMA-accum trees per population: list of
            # (level, tile); merging keeps depth log and liveness low
            pend = {'a': [], 'e': []}

            def _push(key, tile):
                lvl = 0
                st = pend[key]
                while st and st[-1][0] == lvl:
                    _, other = st.pop()
                    nc.gpsimd.dma_start(out=other[:, :], in_=tile[:, :],
                                        accum_op=OP.add)
                    tile = other
                    lvl += 1
                st.append((lvl, tile))

            # enemy-first: the population sums are consumed at tile end
            for s in (3, 4, 5, 0, 1, 2):
                nslots = min(4, NAG - 4 * s)
                slot_order = (2, 3, 0, 1) if s == 2 else tuple(range(nslots))
                for r in slot_order:
                    ag = 4 * s + r
                    isally = ag < A
                    # all 4 hidden chunks of this agent in one tile:
                    # [mlp1-kc0 | mlp1-kc1 | mlp2-kc0 | mlp2-kc1]
                    big = hpool.tile([128, 4 * NF], BF16,
                                     name=f"h_{t}_{ag}", tag="hbig")
                    w4 = w1a4 if isally else w1e4
                    bsrc = b1a_sb if isally else b1e_sb
                    for mc in range(4):
                        pht = ph.tile([128, NF], F32, space="PSUM",
                                      name=f"ph_{t}_{ag}_{mc}", tag="ph")
                        _mm(nc, pht[:, :],
                            w4[ds(32 * r, 32), ds(128 * mc, 128)],
                            stacks[s][ds(32 * r, 32), btsl],
                            start=True, stop=True,
                            tile_position=(32 * r, 0))
                        relu_op(big[:, ds(NF * mc, NF)], pht[:, :],
                                bsrc[:, ds(mc, 1)])
                    if isally:
                        # per-agent w part of layer-1 ally hypernet (pairs)
                        pos = ag % 2
                        if pos == 0:
                            pwa = pt.tile([128, NF], F32, space="PSUM",
                                          name=f"pwa_{t}_{ag}", tag="pt")
                        for kc in range(2):
                            _mm(nc, pwa[ds(E * pos, E), :],
                                w2l1a[:, ds(128 * kc, E)],
                                big[:, ds(NF * kc, NF)],
                                start=(kc == 0), stop=(kc == 1),
                                tile_position=(0, E * pos),
                                skip_group_check=True)
                        if pos == 1:
                            pair = ag // 2
                            qt = qbp.tile([128, NF], F32, name=f"qvb_{t}_{pair}", tag="qvb")
                            nc.sync.dma_start(out=qt[0:E, :], in_=qvB_e[2 * pair, :, btsl])
                            nc.sync.dma_start(out=qt[E:128, :], in_=qvB_e[2 * pair + 1, :, btsl])
                            abs_t = mix.tile([128, NF], F32, name=f"abs_{t}_{pair}", tag="abs")
                            nc.scalar.activation(abs_t[:, :], pwa[:, :], AF.Abs,
                                                 bias=wab_sb)
                            dst = mix.tile([128, NF], F32, name=f"prod_{t}_{pair}",
                                           tag="prod" if hidacc is not None else "hacc")
                            nc.vector.tensor_mul(dst[:, :], abs_t[:, :], qt[:, :])
                            if hidacc is None:
                                hidacc = dst
                            else:
                                nxt = mix.tile([128, NF], F32, name=f"hacc_{t}_{pair}", tag="hacc")
                                nc.gpsimd.tensor_add(nxt[:, :], hidacc[:, :], dst[:, :])
                                hidacc = nxt
                    # running population sum via SDMA accumulate
                    _push('a' if isally else 'e', big)

            def _final(key):
                st = pend[key]
                tile = st.pop()[1]
                while st:
                    _, other = st.pop()
                    nc.gpsimd.dma_start(out=other[:, :], in_=tile[:, :],
                                        accum_op=OP.add)
                    tile = other
                return tile

            re = _final('e')
            ra = _final('a')
            # agent-summed layer-2 contractions on the population sums
            pwe = pt.tile([128, NF], F32, space="PSUM", name=f"pwe_{t}", tag="pt")
            pl2a = pt.tile([E + 1, NF], F32, space="PSUM", name=f"pl2a_{t}", tag="pt")
            pl2e = pt.tile([E + 1, NF], F32, space="PSUM", name=f"pl2e_{t}", tag="pt")
            _mm(nc, pwe[:, :], w2l1e[:, 0:128], re[:, 0:NF],
                start=True, stop=False, skip_group_check=True)
            _mm(nc, pwe[:, :], w2l1e[:, 128:256], re[:, NF:2 * NF],
                start=False, stop=False, skip_group_check=True)
            _mm(nc, pwe[0:E, :], w2l1a[:, 64:128], ra[:, 0:NF],
                start=False, stop=False, skip_group_check=True)
            _mm(nc, pwe[0:E, :], w2l1a[:, 192:256], ra[:, NF:2 * NF],
                start=False, stop=True, skip_group_check=True)
            _mm(nc, pl2a[:, :], w2l2a[:, 0:E + 1], ra[:, 2 * NF:3 * NF],
                start=True, stop=False, skip_group_check=True)
            _mm(nc, pl2a[:, :], w2l2a[:, E + 1:2 * (E + 1)], ra[:, 3 * NF:4 * NF],
                start=False, stop=True, skip_group_check=True)
            _mm(nc, pl2e[:, :], w2l2e[:, 0:E + 1], re[:, 2 * NF:3 * NF],
                start=True, stop=False, skip_group_check=True)
            _mm(nc, pl2e[:, :], w2l2e[:, E + 1:2 * (E + 1)], re[:, 3 * NF:4 * NF],
                start=False, stop=False, skip_group_check=True)

            # ---------------- mixing ----------------
            # qsum broadcast to all 128 partitions (M=128, no col tiling)
            pq = pt.tile([128, NF], F32, space="PSUM", name=f"pq_{t}", tag="pt")
            _mm(nc, pq[:, :], ones10, qvT[:, btsl], start=True, stop=True)
            we_t = mix.tile([128, NF], F32, name=f"we_{t}", tag="we")
            nc.scalar.activation(we_t[E:128, :], pwe[E:128, :], AF.Abs,
                                 bias=web_sb[E:128, :])
            he_t = mix.tile([128, NF], BF16, name=f"he_{t}", tag="he")
            nc.gpsimd.memset(he_t[0:E, :], 0.0)
            nc.vector.tensor_mul(he_t[E:128, :], we_t[E:128, :], pq[E:128, :])
            # fold he down to partitions 0:64 via PE
            hacc16 = mix.tile([128, NF], BF16, name=f"hacc16_{t}", tag="hacc16")
            nc.vector.tensor_copy(hacc16[:, :], hidacc[:, :])
            pf = pt.tile([E, NF], F32, space="PSUM", name=f"pf_{t}", tag="pt")
            _mm(nc, pf[:, :], fmat2, he_t[:, :], start=True, stop=False)
            _mm(nc, pf[:, :], fmat, hacc16[:, :], start=False, stop=True)
            t1 = mix.tile([E, NF], F32, name=f"t1_{t}", tag="t1")
            nc.vector.tensor_scalar(t1[:, :], pf[:, :], zb_sb, None, OP.add)
            z = mix.tile([E, NF], F32, name=f"z_{t}", tag="z")
            nc.vector.tensor_add(z[:, :], t1[:, :], pwe[0:E, :])
            # elu(z) = relu(z) + exp(min(z,0)) - 1
            tmin = mix.tile([E, NF], F32, name=f"tmin_{t}", tag="tmin")
            nc.vector.tensor_scalar_min(tmin[:, :], z[:, :], 0.0)
            texp = mix.tile([E, NF], F32, name=f"texp_{t}", tag="texp")
            nc.scalar.activation(texp[:, :], tmin[:, :], AF.Exp)
            trelu = mix.tile([E, NF], F32, name=f"trelu_{t}", tag="trelu")
            nc.scalar.activation(trelu[:, :], z[:, :], AF.Relu)
            hidden = mix.tile([E, NF], F32, name=f"hidden_{t}", tag="hidden")
            nc.vector.scalar_tensor_tensor(hidden[:, :], texp[:, :], -1.0,
                                           trelu[:, :], OP.add, OP.add)
            w2a_t = mix.tile([E, NF], F32, name=f"w2a_{t}", tag="w2a")
            nc.scalar.activation(w2a_t[:, :], pl2a[0:E, :], AF.Abs,
                                 bias=w2ab_sb)
            w2e_t = mix.tile([E, NF], F32, name=f"w2e_{t}", tag="w2e")
            nc.scalar.activation(w2e_t[:, :], pl2e[0:E, :], AF.Abs,
                                 bias=w2eb_sb)
            w2s = mix.tile([E, NF], F32, name=f"w2s_{t}", tag="w2s")
            nc.vector.tensor_add(w2s[:, :], w2a_t[:, :], w2e_t[:, :])
            prodf = mix.tile([E, NF], BF16, name=f"prodf_{t}", tag="prodf")
            nc.vector.tensor_mul(prodf[:, :], hidden[:, :], w2s[:, :])
            # final dot accumulates into pl2e row 64 (dotsel col 64 = ones,
            # all other cols zero -> rows 0:64 get +0)
            _mm(nc, pl2e[:, :], dotsel, prodf[:, :], start=False, stop=True,
                skip_group_check=True)
            s1 = mix.tile([128, NF], F32, name=f"s1_{t}", tag="s1")
            nc.scalar.activation(s1[E:E + 1, :], pl2a[E:E + 1, :], AF.Identity,
                                 bias=ob_sb[E:E + 1, :])
            o_sb = mix.tile([128, NF], F32, name=f"o_{t}", tag="o")
            nc.vector.tensor_add(o_sb[E:E + 1, :], s1[E:E + 1, :], pl2e[E:E + 1, :])
            nc.sync.dma_start(out=out_e[btsl].unsqueeze(0), in_=o_sb[E:E + 1, :])

    return nc


_BUILT = None


def _get_nc():
    global _BUILT
    if _BUILT is None:
        _BUILT = build()
        _BUILT.finalize()
    return _BUILT


def _prep_in_maps(inputs):
    qv = np.ascontiguousarray(np.asarray(inputs["qvals"], dtype=np.float32)).reshape(BT, A)
    st = np.ascontiguousarray(np.asarray(inputs["states"], dtype=np.float32)).reshape(BT, SD)
    f32 = np.float32
    g = {n: np.asarray(inputs[n], dtype=f32) for n in W_NAMES}
    bias = {n: np.asarray(inputs[n], dtype=f32) for n in
            ["l1a_b1", "l1a_b2", "l1e_b1", "l1e_b2",
             "l2a_b1", "l2a_b2", "l2e_b1", "l2e_b2"]}
    w1a4 = np.tile(np.concatenate([g["l1a_w1"], g["l2a_w1"]], axis=1), (4, 1))
    w1e4 = np.tile(np.concatenate([g["l1e_w1"], g["l2e_w1"]], axis=1), (4, 1))
    w2l1a = np.concatenate([g["l1a_w2"][0:128], g["l1a_w2"][128:256]], axis=1)
    # enemy layer-1 W2 with output cols reordered to [b | w]
    l1e_bw = np.concatenate([g["l1e_w2"][:, E:], g["l1e_w2"][:, :E]], axis=1)
    w2l1e = np.concatenate([l1e_bw[0:128], l1e_bw[128:256]], axis=1)
    w2l2a = np.concatenate([g["l2a_w2"][0:128], g["l2a_w2"][128:256]], axis=1)
    w2l2e = np.concatenate([g["l2e_w2"][0:128], g["l2e_w2"][128:256]], axis=1)
    bvec = np.zeros((128, 14), f32)
    bvec[:, 0:4] = np.concatenate([bias["l1a_b1"], bias["l2a_b1"]]).reshape(4, 128).T
    bvec[:, 4:8] = np.concatenate([bias["l1e_b1"], bias["l2e_b1"]]).reshape(4, 128).T
    bvec[0:E, 8] = bias["l1a_b2"][:E]
    bvec[E:128, 8] = bias["l1a_b2"][:E]
    bvec[0:E, 9] = A * bias["l1a_b2"][E:] + NE * bias["l1e_b2"][E:]
    bvec[E:128, 10] = NE * bias["l1e_b2"][:E]
    bvec[0:E, 11] = A * bias["l2a_b2"][:E]
    bvec[0:E, 12] = NE * bias["l2e_b2"][:E]
    bvec[E, 13] = A * bias["l2a_b2"][E] + NE * bias["l2e_b2"][E]
    cmat = np.zeros((128, 321), f32)
    cmat[E:128, 0:E] = np.eye(E, dtype=f32)       # fmat2
    cmat[0:E, E:2 * E] = np.eye(E, dtype=f32)     # fmat (both halves = I)
    cmat[E:128, E:2 * E] = np.eye(E, dtype=f32)
    cmat[0:A, 128:256] = 1.0                      # ones10
    cmat[0:E, 256 + E] = 1.0                      # dotsel col 64
    import ml_dtypes
    bf16 = ml_dtypes.bfloat16
    wmaps = {
        "w1a4": np.ascontiguousarray(w1a4).astype(bf16),
        "w1e4": np.ascontiguousarray(w1e4).astype(bf16),
        "w2l1a": np.ascontiguousarray(w2l1a).astype(bf16),
        "w2l1e": np.ascontiguousarray(w2l1e).astype(bf16),
        "w2l2a": np.ascontiguousarray(w2l2a).astype(bf16),
        "w2l2e": np.ascontiguousarray(w2l2e).astype(bf16),
        "bvec": bvec, "cmat": cmat.astype(bf16),
    }
    in_maps = []
    for c in range(NCORES):
        sl = slice(c * NBT, (c + 1) * NBT)
        qvc = np.ascontiguousarray(qv[sl].T)            # [A, NBT]
        m = {
            "qvT": qvc.astype(bf16),
            "qvB": np.ascontiguousarray(
                np.broadcast_to(qvc[:, None, :], (A, E, NBT))),
            "statesT": np.ascontiguousarray(st[sl].T).astype(bf16),
        }
        m.update(wmaps)
        in_maps.append(m)
    return in_maps


def run(inputs, **kw):
    nc = _get_nc()
    in_maps = _prep_in_maps(inputs)
    res = run_bass_kernel_spmd(nc, in_maps, list(range(NCORES)), **kw)
    out = np.concatenate([
        np.asarray(res.results[i]["out"], dtype=np.float32).reshape(NBT)
        for i in range(NCORES)])
    return out.reshape(B, T, 1), res


def kernel(**inputs):
    out, _ = run(inputs)
    return out
